# revision 1
# baseline (speedup 1.0000x reference)
"""Trainium2 Bass kernel for nn_CrossAttention (B=2, Lq=Lkv=2048, E=1024, H=16, D=64).

Sharding: tensor-parallel over heads. Each of the 8 cores owns 2 heads
(a 128-wide slice of the QKV projection output and the matching 128
columns of Wo). Per core:

  phase P: Q^T/K^T/V^T projections (contraction over E in 8 chunks of
           128, fp32r matmuls at full PE rate), biases fused into the
           PSUM->SBUF copy on ScalarE.
  phase T: V^T -> V via PE transposes; V stored as [kpart, chunk,
           [1|h0|1|h1]] so a ones column rides along as stationary
           column 0, making each context matmul also produce the
           softmax denominator in PSUM row 0.
  phase A: flash-style attention per (batch, 512-wide q tile):
           scores^T = K^T.T @ Q^T with 64-row PE tiling (head0 on
           partitions 0-63, head1 on 64-127, concurrent); exp+mask via
           one ScalarE activation (scale=1/8, per-partition additive
           mask bias) straight from PSUM; context accumulated over the
           16 k chunks into 4 PSUM banks (2 row-tiles x 2 heads);
           denominator division via reciprocal + K=1 broadcast matmul;
           SBUF->SBUF DMA assembles ctx into [128 j, t] layout.
  phase O: out^T partial = Wo_c^T.T @ ctx, written to DRAM; host sums
           the 8 partials (the row-parallel Wo all-reduce).
"""

import sys

if "/opt/trn_rl_repo" not in sys.path:
    sys.path.insert(0, "/opt/trn_rl_repo")

import numpy as np

import concourse.tile as tile
from concourse import bacc, mybir
from concourse.bass_utils import run_bass_kernel_spmd
from concourse.masks import make_identity

F32 = mybir.dt.float32
F32R = mybir.dt.float32r
AF = mybir.ActivationFunctionType

N_CORES = 8
B, LQ, LKV, E, H, D = 2, 2048, 2048, 1024, 16, 64
HC = H // N_CORES  # heads per core = 2
JC = HC * D  # feature slice per core = 128
T = B * LQ  # 4096 tokens
NEC = E // 128  # 8 e-chunks
NTT = T // 512  # 8 token tiles of 512
NQT = LQ // 512  # 4 q tiles per batch
NKT = LKV // 128  # 16 k chunks per batch
NOC = E // 128  # 8 output chunks

_NC_CACHE = {}


def build(reps=None, phases="PTAO"):
    key = (reps or 0, phases)
    if key in _NC_CACHE:
        return _NC_CACHE[key]
    nc = bacc.Bacc("TRN2", target_bir_lowering=False, debug=False, num_devices=N_CORES)

    xqT = nc.dram_tensor("xqT", [E, T], F32R, kind="ExternalInput").ap()
    xkT = nc.dram_tensor("xkT", [E, T], F32R, kind="ExternalInput").ap()
    wqT = nc.dram_tensor("wqT", [E, JC], F32R, kind="ExternalInput").ap()
    wkT = nc.dram_tensor("wkT", [E, JC], F32R, kind="ExternalInput").ap()
    wvT = nc.dram_tensor("wvT", [E, JC], F32R, kind="ExternalInput").ap()
    woT = nc.dram_tensor("woT", [JC, E], F32R, kind="ExternalInput").ap()
    bqd = nc.dram_tensor("bq", [JC, 1], F32, kind="ExternalInput").ap()
    bkd = nc.dram_tensor("bk", [JC, 1], F32, kind="ExternalInput").ap()
    bvd = nc.dram_tensor("bv", [JC, 1], F32, kind="ExternalInput").ap()
    bod = nc.dram_tensor("bo", [NOC, 128], F32, kind="ExternalInput").ap()
    mbd = nc.dram_tensor("mb", [B, NKT, 128], F32, kind="ExternalInput").ap()
    outT = nc.dram_tensor("outT", [E, T], F32, kind="ExternalOutput").ap()

    from contextlib import nullcontext

    with tile.TileContext(nc) as tc, nc.allow_low_precision(reason="fp32r matmuls"):
        with tc.For_i(0, reps, 1) if reps else nullcontext():
         with (
             tc.tile_pool(name="const", bufs=1) as const,
             tc.tile_pool(name="big", bufs=1) as big,
         ):
             # ---- persistent SBUF state ----
             wq_sb = const.tile([128, NEC, JC], F32R, tag="wq")
             nc.sync.dma_start(out=wq_sb, in_=wqT.rearrange("(ec p) j -> p ec j", p=128))
             wk_sb = const.tile([128, NEC, JC], F32R, tag="wk")
             nc.sync.dma_start(out=wk_sb, in_=wkT.rearrange("(ec p) j -> p ec j", p=128))
             wv_sb = const.tile([128, NEC, JC], F32R, tag="wv")
             nc.sync.dma_start(out=wv_sb, in_=wvT.rearrange("(ec p) j -> p ec j", p=128))
             wo_sb = const.tile([128, NOC, 128], F32R, tag="wo")
             nc.sync.dma_start(out=wo_sb, in_=woT.rearrange("p (oc o) -> p oc o", oc=NOC))
             bq_sb = const.tile([128, 1], F32, tag="bq")
             nc.sync.dma_start(out=bq_sb, in_=bqd)
             bk_sb = const.tile([128, 1], F32, tag="bk")
             nc.sync.dma_start(out=bk_sb, in_=bkd)
             bv_sb = const.tile([128, 1], F32, tag="bv")
             nc.sync.dma_start(out=bv_sb, in_=bvd)
             bo_sb = const.tile([128, NOC], F32, tag="bo")
             nc.sync.dma_start(out=bo_sb, in_=bod.rearrange("oc o -> o oc"))
             mb_sb = const.tile([128, B, NKT], F32, tag="mb")
             nc.sync.dma_start(out=mb_sb, in_=mbd.rearrange("b kc p -> p b kc"))
             ident = const.tile([128, 128], F32, tag="ident")
             make_identity(nc, ident)
             ones_f = const.tile([1, 65], F32, tag="onesf")
             nc.vector.memset(ones_f, 1.0)
             onesc = const.tile([1, 65], F32R, tag="onesc")
             nc.vector.tensor_copy(onesc, ones_f)
             onecol = const.tile([128, 1], F32, tag="onecol")
             nc.vector.memset(onecol, 1.0)

             qt_sb = big.tile([128, T], F32R, tag="qt")
             kt_sb = big.tile([128, T], F32R, tag="kt")
             vt_sb = big.tile([128, T], F32, tag="vt")
             v_sb = big.tile([128, B * NKT, 130], F32R, tag="v")
             ctx_sb = big.tile([128, NTT, 512], F32R, tag="ctx")

             # ---- phase P: projections ----
             if "P" in phases:
              with (
                 tc.tile_pool(name="xin", bufs=2) as xin,
                 tc.tile_pool(name="pp", bufs=3, space="PSUM") as pp,
             ):
                 for xsrc, wsb, bias, dst, isv in (
                     (xqT, wq_sb, bq_sb, qt_sb, False),
                     (xkT, wk_sb, bk_sb, kt_sb, False),
                     (xkT, wv_sb, bv_sb, vt_sb, True),
                 ):
                     for tt in range(NTT):
                         xt = xin.tile([128, NEC, 512], F32R, tag="xin")
                         nc.sync.dma_start(
                             out=xt,
                             in_=xsrc[:, tt * 512 : (tt + 1) * 512].rearrange(
                                 "(ec p) t -> p ec t", p=128
                             ),
                         )
                         pt = pp.tile([128, 512], F32, tag="pp")
                         for ec in range(NEC):
                             nc.tensor.matmul(
                                 pt,
                                 wsb[:, ec, :],
                                 xt[:, ec, :],
                                 start=(ec == 0),
                                 stop=(ec == NEC - 1),
                             )
                         nc.scalar.activation(
                             out=dst[:, tt * 512 : (tt + 1) * 512],
                             in_=pt,
                             func=AF.Identity,
                             bias=bias,
                             scale=1.0,
                         )

             # ---- phase T: V transpose into [kpart, chunk, [1|h0|1|h1]] ----
             if "T" in phases:
              with tc.tile_pool(name="tp", bufs=3, space="PSUM") as tp:
                 for gc in range(B * NKT):
                     tpt = tp.tile([128, 128], F32, tag="tp")
                     nc.tensor.transpose(
                         tpt, vt_sb[:, gc * 128 : (gc + 1) * 128], ident
                     )
                     nc.vector.tensor_copy(v_sb[:, gc, 1:65], tpt[:, 0:64])
                     nc.vector.tensor_copy(v_sb[:, gc, 66:130], tpt[:, 64:128])
                     nc.vector.tensor_copy(v_sb[:, gc, 0:1], onecol)
                     nc.vector.tensor_copy(v_sb[:, gc, 65:66], onecol)

             # ---- phase A: attention ----
             if "A" in phases:
              with (
                 tc.tile_pool(name="attps", bufs=2, space="PSUM") as attps,
                 tc.tile_pool(name="cxps", bufs=1, space="PSUM") as cxps,
                 tc.tile_pool(name="expm", bufs=3) as expm,
                 tc.tile_pool(name="dv", bufs=2) as dv,
             ):
                 for b in range(B):
                     for qt in range(NQT):
                         q0 = b * LQ + qt * 512
                         cxs = [
                             cxps.tile([65, 512], F32, tag=f"cx{i}", name=f"cx{i}_{b}_{qt}")
                             for i in range(4)
                         ]
                         for kt in range(NKT):
                             k0 = b * LKV + kt * 128
                             sct = attps.tile([128, 2, 512], F32, tag="sc")
                             nc.tensor.matmul(
                                 sct[:, 0, :],
                                 kt_sb[0:64, k0 : k0 + 128],
                                 qt_sb[0:64, q0 : q0 + 512],
                                 start=True,
                                 stop=True,
                             )
                             nc.tensor.matmul(
                                 sct[:, 1, :],
                                 kt_sb[64:128, k0 : k0 + 128],
                                 qt_sb[64:128, q0 : q0 + 512],
                                 start=True,
                                 stop=True,
                             )
                             emt = expm.tile([128, 2, 512], F32R, tag="expm")
                             nc.scalar.activation(
                                 out=emt.rearrange("p a t -> p (a t)"),
                                 in_=sct.rearrange("p a t -> p (a t)"),
                                 func=AF.Exp,
                                 bias=mb_sb[:, b, kt : kt + 1],
                                 scale=0.125,
                             )
                             st, sp = (kt == 0), (kt == NKT - 1)
                             gc = b * NKT + kt
                             nc.tensor.matmul(
                                 cxs[0], v_sb[0:64, gc, 0:65], emt[0:64, 0, :],
                                 start=st, stop=sp,
                             )
                             nc.tensor.matmul(
                                 cxs[1], v_sb[64:128, gc, 0:65], emt[64:128, 0, :],
                                 start=st, stop=sp,
                             )
                             nc.tensor.matmul(
                                 cxs[2], v_sb[0:64, gc, 65:130], emt[0:64, 1, :],
                                 start=st, stop=sp,
                             )
                             nc.tensor.matmul(
                                 cxs[3], v_sb[64:128, gc, 65:130], emt[64:128, 1, :],
                                 start=st, stop=sp,
                             )
                         tt = b * NQT + qt
                         for h in range(HC):
                             cxa, cxb = cxs[2 * h], cxs[2 * h + 1]
                             s1 = dv.tile([65, 512], F32, tag="s1")
                             nc.vector.tensor_copy(s1, cxa)
                             s2 = dv.tile([65, 512], F32, tag="s2")
                             nc.vector.tensor_add(s2, s1, cxb)
                             rr = dv.tile([1, 512], F32R, tag="rr")
                             nc.vector.reciprocal(rr, s2[0:1, :])
                             s2r = dv.tile([65, 512], F32R, tag="s2r")
                             nc.vector.tensor_copy(s2r, s2)
                             bct = attps.tile([65, 512], F32, tag="sc")
                             nc.tensor.matmul(bct, onesc, rr, start=True, stop=True)
                             cs = dv.tile([65, 512], F32R, tag="cs")
                             nc.vector.tensor_mul(cs, s2r, bct)
                             nc.sync.dma_start(
                                 out=ctx_sb[h * 64 : (h + 1) * 64, tt, :],
                                 in_=cs[1:65, :],
                             )

             # ---- phase O: output projection (partial; host sums cores) ----
             if "O" in phases:
              with (
                 tc.tile_pool(name="ops", bufs=3, space="PSUM") as ops,
                 tc.tile_pool(name="outsb", bufs=3) as outsb,
             ):
                 for tt in range(NTT):
                     for oc in range(NOC):
                         opt = ops.tile([128, 512], F32, tag="op")
                         nc.tensor.matmul(
                             opt, wo_sb[:, oc, :], ctx_sb[:, tt, :],
                             start=True, stop=True,
                         )
                         ob = outsb.tile([128, 512], F32, tag="ob")
                         nc.vector.tensor_scalar_add(ob, opt, bo_sb[:, oc : oc + 1])
                         nc.sync.dma_start(
                             out=outT[oc * 128 : (oc + 1) * 128, tt * 512 : (tt + 1) * 512],
                             in_=ob,
                         )

    nc.compile()
    _NC_CACHE[key] = nc
    return nc


def make_in_maps(query, key_value, mask, Wq, bq, Wk, bk, Wv, bv, Wo, bo):
    xqT = np.ascontiguousarray(query.reshape(T, E).T).astype(np.float32)
    xkT = np.ascontiguousarray(key_value.reshape(T, E).T).astype(np.float32)
    mb = np.where(mask != 0, 0.0, -1.0e5).astype(np.float32).reshape(B, NKT, 128)
    in_maps = []
    for c in range(N_CORES):
        sl = slice(c * JC, (c + 1) * JC)
        in_maps.append(
            {
                "xqT": xqT,
                "xkT": xkT,
                "wqT": np.ascontiguousarray(Wq[sl, :].T).astype(np.float32),
                "wkT": np.ascontiguousarray(Wk[sl, :].T).astype(np.float32),
                "wvT": np.ascontiguousarray(Wv[sl, :].T).astype(np.float32),
                "woT": np.ascontiguousarray(Wo[:, sl].T).astype(np.float32),
                "bq": bq[sl].reshape(JC, 1).astype(np.float32),
                "bk": bk[sl].reshape(JC, 1).astype(np.float32),
                "bv": bv[sl].reshape(JC, 1).astype(np.float32),
                # only core 0 adds bo so the host-side partial sum sees it once
                "bo": (
                    bo.reshape(NOC, 128).astype(np.float32)
                    if c == 0
                    else np.zeros((NOC, 128), np.float32)
                ),
                "mb": mb,
            }
        )
    return in_maps


def kernel(query, key_value, mask, Wq, bq, Wk, bk, Wv, bv, Wo, bo):
    nc = build()
    in_maps = make_in_maps(
        np.asarray(query), np.asarray(key_value), np.asarray(mask),
        np.asarray(Wq), np.asarray(bq), np.asarray(Wk), np.asarray(bk),
        np.asarray(Wv), np.asarray(bv), np.asarray(Wo), np.asarray(bo),
    )
    res = run_bass_kernel_spmd(nc, in_maps, list(range(N_CORES)))
    acc = np.zeros((E, T), np.float32)
    for c in range(N_CORES):
        acc += res.results[c]["outT"]
    return np.ascontiguousarray(acc.T).reshape(B, LQ, E).astype(np.float32)



# revision 5
# speedup vs baseline: 2.1601x; 2.1601x over previous
"""Trainium2 Bass kernel for nn_CrossAttention (B=2, Lq=Lkv=2048, E=1024, H=16, D=64).

Sharding: tensor-parallel over heads. Each of the 8 cores owns 2 heads
(a 128-wide slice of the QKV projection output and the matching 128
columns of Wo); the host sums the 8 partial outputs (the row-parallel
Wo all-reduce).

Key optimizations over the v1 kernel:
  - The kv positions with mask==0 are removed on the HOST (exact math:
    softmax assigns them probability 0). The kernel is specialized per
    (valid-count) tuple and cached; scores/exp/context/KV-projection all
    shrink by the mask density (~2x for a random 0/1 mask).
  - bf16 for x, weights, probs, V, ctx and the output partial: halves
    all DMA traffic and SBUF footprint at full PE rate.
  - V is produced directly in [k, d] layout by a flipped matmul
    (x chunk stationary, Wv moving) -- no separate transpose phase.
    bv rides in via a K=1 ones-row matmul into the same PSUM tile.
  - Context matmuls contract the full 128 k-rows of a chunk (v1 split
    them into 2x64 and paid double the PE streaming).
  - A ones column in the V stationary makes each context matmul also
    accumulate the softmax denominator in PSUM row 0.
  - key_value is read from HBM once (v1 read it twice), the output
    partial is written bf16 in 2KB lines, bo is added on the host.
"""

import sys

if "/opt/trn_rl_repo" not in sys.path:
    sys.path.insert(0, "/opt/trn_rl_repo")

import numpy as np
import ml_dtypes

import concourse.tile as tile
from concourse import bacc, mybir
from concourse.bass_utils import run_bass_kernel_spmd

F32 = mybir.dt.float32
F32R = mybir.dt.float32r
BF16 = mybir.dt.bfloat16
AF = mybir.ActivationFunctionType
BF16NP = ml_dtypes.bfloat16

N_CORES = 8
B, LQ, LKV, E, H, D = 2, 2048, 2048, 1024, 16, 64
HC = H // N_CORES  # heads per core = 2
JC = HC * D  # feature slice per core = 128
T = B * LQ  # 4096 query tokens
NEC = E // 128  # 8 e-chunks
NQT = LQ // 512  # 4 q tiles per batch
NTT = B * NQT  # 8 token tiles of 512
NOC = E // 128  # 8 output chunks

_NC_CACHE = {}
_LAST_META = None


def build(reps=None, phases="PAO", meta=None):
    """meta = (nkt0, nkt1): number of 128-wide valid kv chunks per batch."""
    global _LAST_META
    if meta is None:
        meta = _LAST_META
    assert meta is not None, "call make_in_maps first (sets kv chunk counts)"
    nkt = meta
    key = (reps or 0, phases, meta)
    if key in _NC_CACHE:
        return _NC_CACHE[key]
    # 512-wide projection tiles per batch; attention iterates only valid chunks
    nkv5 = tuple(-(-n * 128 // 512) for n in nkt)  # ceil(nkt*128/512)
    kvw = (nkv5[0] + nkv5[1]) * 512  # packed kv width incl. padding
    kvoff = (0, nkv5[0] * 512)  # kt_sb column offset per batch
    NG = nkt[0] + nkt[1]  # total valid kv chunks
    gcoff = (0, nkt[0])

    nc = bacc.Bacc("TRN2", target_bir_lowering=False, debug=False, num_devices=N_CORES)

    xqT = nc.dram_tensor("xqT", [E, T], BF16, kind="ExternalInput").ap()
    xkT = nc.dram_tensor("xkT", [E, kvw], BF16, kind="ExternalInput").ap()
    wqT = nc.dram_tensor("wqT", [E, JC], BF16, kind="ExternalInput").ap()
    wkT = nc.dram_tensor("wkT", [E, JC], BF16, kind="ExternalInput").ap()
    wvT = nc.dram_tensor("wvT", [E, JC], BF16, kind="ExternalInput").ap()
    woT = nc.dram_tensor("woT", [JC, E], BF16, kind="ExternalInput").ap()
    bqd = nc.dram_tensor("bq", [JC, 1], F32, kind="ExternalInput").ap()
    bkd = nc.dram_tensor("bk", [JC, 1], F32, kind="ExternalInput").ap()
    bvd = nc.dram_tensor("bv", [1, JC], BF16, kind="ExternalInput").ap()
    mbd = nc.dram_tensor("mb", [128, NG], F32, kind="ExternalInput").ap()
    outT = nc.dram_tensor("outT", [E, T], BF16, kind="ExternalOutput").ap()

    from contextlib import nullcontext

    with tile.TileContext(nc) as tc, nc.allow_low_precision(reason="bf16 kernel"):
        with tc.For_i(0, reps, 1) if reps else nullcontext():
         with (
             tc.tile_pool(name="const", bufs=1) as const,
             tc.tile_pool(name="big", bufs=1) as big,
         ):
             # ---- persistent SBUF state ----
             wq_sb = const.tile([128, NEC, JC], BF16, tag="wq")
             nc.sync.dma_start(out=wq_sb, in_=wqT.rearrange("(ec p) j -> p ec j", p=128))
             wk_sb = const.tile([128, NEC, JC], BF16, tag="wk")
             nc.sync.dma_start(out=wk_sb, in_=wkT.rearrange("(ec p) j -> p ec j", p=128))
             wv_sb = const.tile([128, NEC, JC], BF16, tag="wv")
             nc.sync.dma_start(out=wv_sb, in_=wvT.rearrange("(ec p) j -> p ec j", p=128))
             wo_sb = const.tile([128, NOC, 128], BF16, tag="wo")
             nc.sync.dma_start(out=wo_sb, in_=woT.rearrange("p (oc o) -> p oc o", oc=NOC))
             bq_sb = const.tile([128, 1], F32, tag="bq")
             nc.sync.dma_start(out=bq_sb, in_=bqd)
             bk_sb = const.tile([128, 1], F32, tag="bk")
             nc.sync.dma_start(out=bk_sb, in_=bkd)
             bv_sb = const.tile([1, JC], BF16, tag="bv")
             nc.sync.dma_start(out=bv_sb, in_=bvd)
             mb_sb = const.tile([128, NG], F32, tag="mb")
             nc.sync.dma_start(out=mb_sb, in_=mbd)
             ones1 = const.tile([1, 128], BF16, tag="ones1")
             nc.vector.memset(ones1, 1.0)
             ones_f = const.tile([1, 65], F32, tag="onesf")
             nc.vector.memset(ones_f, 1.0)
             onesc = const.tile([1, 65], F32R, tag="onesc")
             nc.vector.tensor_copy(onesc, ones_f)

             qt_sb = big.tile([128, T], BF16, tag="qt")
             kt_sb = big.tile([128, kvw], BF16, tag="kt")
             # V per chunk: [1 | h0 d0..63 | 1 | h1 d0..63]
             v_sb = big.tile([128, NG, 130], BF16, tag="v")
             nc.vector.memset(v_sb[:, :, 0:1], 1.0)
             nc.vector.memset(v_sb[:, :, 65:66], 1.0)
             ctx_sb = big.tile([128, NTT, 512], BF16, tag="ctx")

             # ---- phase P: projections ----
             if "P" in phases:
              with (
                 tc.tile_pool(name="xin", bufs=3) as xin,
                 tc.tile_pool(name="pp", bufs=2, space="PSUM") as pp,
                 tc.tile_pool(name="vp", bufs=4, space="PSUM") as vp,
             ):
                 # K^T and V from packed kv (single read of xkT)
                 for b in range(B):
                     for t5 in range(nkv5[b]):
                         toff = kvoff[b] + t5 * 512
                         xt = xin.tile([128, NEC, 512], BF16, tag="xin")
                         nc.sync.dma_start(
                             out=xt,
                             in_=xkT[:, toff : toff + 512].rearrange(
                                 "(ec p) t -> p ec t", p=128
                             ),
                         )
                         pt = pp.tile([128, 512], F32, tag="pp")
                         for ec in range(NEC):
                             nc.tensor.matmul(
                                 pt,
                                 wk_sb[:, ec, :],
                                 xt[:, ec, :],
                                 start=(ec == 0),
                                 stop=(ec == NEC - 1),
                             )
                         nc.scalar.activation(
                             out=kt_sb[:, toff : toff + 512],
                             in_=pt,
                             func=AF.Identity,
                             bias=bk_sb,
                             scale=1.0,
                         )
                         # V chunks within this 512-token tile, [k, d] layout
                         for ck in range(4):
                             kt_g = t5 * 4 + ck
                             if kt_g >= nkt[b]:
                                 break
                             gc = gcoff[b] + kt_g
                             vt = vp.tile([128, 128], F32, tag="vp")
                             for ec in range(NEC):
                                 nc.tensor.matmul(
                                     vt,
                                     xt[:, ec, ck * 128 : (ck + 1) * 128],
                                     wv_sb[:, ec, :],
                                     start=(ec == 0),
                                     stop=False,
                                 )
                             nc.tensor.matmul(
                                 vt, ones1, bv_sb, start=False, stop=True
                             )
                             # cols (1:65, 66:130) <- PSUM cols 0:128
                             nc.vector.tensor_copy(
                                 v_sb[:, gc, :].rearrange(
                                     "p (a d) -> p a d", d=65
                                 )[:, :, 1:65],
                                 vt.rearrange("p (a d) -> p a d", a=2),
                             )

                 # Q^T projection
                 for tt in range(NTT):
                     xt = xin.tile([128, NEC, 512], BF16, tag="xin")
                     nc.sync.dma_start(
                         out=xt,
                         in_=xqT[:, tt * 512 : (tt + 1) * 512].rearrange(
                             "(ec p) t -> p ec t", p=128
                         ),
                     )
                     pt = pp.tile([128, 512], F32, tag="pp")
                     for ec in range(NEC):
                         nc.tensor.matmul(
                             pt,
                             wq_sb[:, ec, :],
                             xt[:, ec, :],
                             start=(ec == 0),
                             stop=(ec == NEC - 1),
                         )
                     nc.scalar.activation(
                         out=qt_sb[:, tt * 512 : (tt + 1) * 512],
                         in_=pt,
                         func=AF.Identity,
                         bias=bq_sb,
                         scale=1.0,
                     )

             # ---- phase A: attention ----
             if "A" in phases:
              with (
                 tc.tile_pool(name="attps", bufs=2, space="PSUM") as attps,
                 tc.tile_pool(name="cxps", bufs=1, space="PSUM") as cxps,
                 tc.tile_pool(name="expm", bufs=3) as expm,
                 tc.tile_pool(name="dv", bufs=2) as dv,
             ):
                 for b in range(B):
                     for qt in range(NQT):
                         q0 = b * LQ + qt * 512
                         tt = b * NQT + qt
                         cxs = [
                             cxps.tile([65, 512], F32, tag=f"cx{h}", name=f"cx{h}_{b}_{qt}")
                             for h in range(HC)
                         ]
                         for kt in range(nkt[b]):
                             k0 = kvoff[b] + kt * 128
                             gc = gcoff[b] + kt
                             sct = attps.tile([128, 2, 512], F32, tag="sc")
                             for h in range(HC):
                                 nc.tensor.matmul(
                                     sct[:, h, :],
                                     kt_sb[h * 64 : (h + 1) * 64, k0 : k0 + 128],
                                     qt_sb[h * 64 : (h + 1) * 64, q0 : q0 + 512],
                                     start=True,
                                     stop=True,
                                 )
                             emt = expm.tile([128, 2, 512], BF16, tag="expm")
                             nc.scalar.activation(
                                 out=emt.rearrange("p a t -> p (a t)"),
                                 in_=sct.rearrange("p a t -> p (a t)"),
                                 func=AF.Exp,
                                 bias=mb_sb[:, gc : gc + 1],
                                 scale=0.125,
                             )
                             st, sp = (kt == 0), (kt == nkt[b] - 1)
                             for h in range(HC):
                                 nc.tensor.matmul(
                                     cxs[h],
                                     v_sb[:, gc, h * 65 : (h + 1) * 65],
                                     emt[:, h, :],
                                     start=st,
                                     stop=sp,
                                 )
                         for h in range(HC):
                             cx = cxs[h]
                             rr = dv.tile([1, 512], F32R, tag="rr")
                             nc.vector.reciprocal(rr, cx[0:1, :])
                             s2 = dv.tile([65, 512], F32R, tag="s2")
                             nc.vector.tensor_copy(s2, cx)
                             bct = attps.tile([65, 512], F32, tag="sc")
                             nc.tensor.matmul(bct, onesc, rr, start=True, stop=True)
                             cs = dv.tile([65, 512], BF16, tag="cs")
                             nc.vector.tensor_mul(cs, s2, bct)
                             nc.sync.dma_start(
                                 out=ctx_sb[h * 64 : (h + 1) * 64, tt, :],
                                 in_=cs[1:65, :],
                             )

             # ---- phase O: output projection (partial; host sums cores) ----
             if "O" in phases:
              with (
                 tc.tile_pool(name="ops", bufs=2, space="PSUM") as ops,
                 tc.tile_pool(name="outsb", bufs=3) as outsb,
             ):
                 for tt2 in range(NTT // 2):
                     for oc in range(NOC):
                         ob = outsb.tile([128, 2, 512], BF16, tag="ob")
                         for half in range(2):
                             tt = tt2 * 2 + half
                             opt = ops.tile([128, 512], F32, tag="op")
                             nc.tensor.matmul(
                                 opt, wo_sb[:, oc, :], ctx_sb[:, tt, :],
                                 start=True, stop=True,
                             )
                             if (oc + half) % 2 == 0:
                                 nc.scalar.activation(
                                     out=ob[:, half, :], in_=opt,
                                     func=AF.Identity, scale=1.0,
                                 )
                             else:
                                 nc.vector.tensor_copy(ob[:, half, :], opt)
                         nc.sync.dma_start(
                             out=outT[
                                 oc * 128 : (oc + 1) * 128,
                                 tt2 * 1024 : (tt2 + 1) * 1024,
                             ],
                             in_=ob.rearrange("p a t -> p (a t)"),
                         )

    nc.compile()
    _NC_CACHE[key] = nc
    return nc


def make_in_maps(query, key_value, mask, Wq, bq, Wk, bk, Wv, bv, Wo, bo):
    global _LAST_META
    # pack valid kv positions per batch (mask==0 rows contribute exactly 0)
    idx = [np.nonzero(mask[b] != 0)[0] for b in range(B)]
    cnt = [len(i) for i in idx]
    nkt = tuple(-(-c // 128) for c in cnt)  # valid 128-chunks per batch
    nkv5 = tuple(-(-n * 128 // 512) for n in nkt)
    _LAST_META = nkt
    NG = nkt[0] + nkt[1]

    xq = np.ascontiguousarray(query.reshape(T, E).T).astype(BF16NP)
    packs = []
    for b in range(B):
        xb = np.zeros((nkv5[b] * 512, E), np.float32)
        xb[: cnt[b]] = key_value[b][idx[b]]
        packs.append(xb)
    xk = np.ascontiguousarray(np.concatenate(packs, axis=0).T).astype(BF16NP)
    # mask bias per chunk lane: 0 for valid, -1e5 for pad
    mb = np.full((128, NG), -1.0e5, np.float32)
    g = 0
    for b in range(B):
        for k in range(nkt[b]):
            valid = min(cnt[b] - k * 128, 128)
            mb[:valid, g] = 0.0
            g += 1

    in_maps = []
    for c in range(N_CORES):
        sl = slice(c * JC, (c + 1) * JC)
        in_maps.append(
            {
                "xqT": xq,
                "xkT": xk,
                "wqT": np.ascontiguousarray(Wq[sl, :].T).astype(BF16NP),
                "wkT": np.ascontiguousarray(Wk[sl, :].T).astype(BF16NP),
                "wvT": np.ascontiguousarray(Wv[sl, :].T).astype(BF16NP),
                "woT": np.ascontiguousarray(Wo[:, sl].T).astype(BF16NP),
                "bq": bq[sl].reshape(JC, 1).astype(np.float32),
                "bk": bk[sl].reshape(JC, 1).astype(np.float32),
                "bv": bv[sl].reshape(1, JC).astype(BF16NP),
                "mb": mb,
            }
        )
    return in_maps


def kernel(query, key_value, mask, Wq, bq, Wk, bk, Wv, bv, Wo, bo):
    in_maps = make_in_maps(
        np.asarray(query), np.asarray(key_value), np.asarray(mask),
        np.asarray(Wq), np.asarray(bq), np.asarray(Wk), np.asarray(bk),
        np.asarray(Wv), np.asarray(bv), np.asarray(Wo), np.asarray(bo),
    )
    nc = build()
    res = run_bass_kernel_spmd(nc, in_maps, list(range(N_CORES)))
    acc = np.zeros((E, T), np.float32)
    for c in range(N_CORES):
        acc += res.results[c]["outT"].astype(np.float32)
    out = np.ascontiguousarray(acc.T).reshape(B, LQ, E)
    out += np.asarray(bo, np.float32)[None, None, :]
    return out.astype(np.float32)


# revision 21
# speedup vs baseline: 3.4892x; 1.6153x over previous
"""Trainium2 Bass kernel for nn_CrossAttention (B=2, Lq=Lkv=2048, E=1024, H=16, D=64).

Sharding: tensor-parallel over heads. Each of the 8 cores owns 2 heads
(a 128-wide slice of the QKV projection output and the matching 128
columns of Wo); the host sums the 8 partial outputs (the row-parallel
Wo all-reduce).

Key optimizations over the v1 kernel:
  - The kv positions with mask==0 are removed on the HOST (exact math:
    softmax assigns them probability 0). The kernel is specialized per
    (valid-count) tuple and cached; scores/exp/context/KV-projection all
    shrink by the mask density (~2x for a random 0/1 mask).
  - bf16 for x, weights, probs, V, ctx and the output partial: halves
    all DMA traffic and SBUF footprint at full PE rate.
  - V is produced directly in [k, d] layout by a flipped matmul
    (x chunk stationary, Wv moving) -- no separate transpose phase.
    bv rides in via a K=1 ones-row matmul into the same PSUM tile.
  - Context matmuls contract the full 128 k-rows of a chunk (v1 split
    them into 2x64 and paid double the PE streaming).
  - A ones column in the V stationary makes each context matmul also
    accumulate the softmax denominator in PSUM row 0.
  - key_value is read from HBM once (v1 read it twice), the output
    partial is written bf16 in 2KB lines, bo is added on the host.
"""

import sys

if "/opt/trn_rl_repo" not in sys.path:
    sys.path.insert(0, "/opt/trn_rl_repo")

import numpy as np
import ml_dtypes

import concourse.tile as tile
from concourse import bacc, mybir
from concourse.bass_utils import run_bass_kernel_spmd

F32 = mybir.dt.float32
F32R = mybir.dt.float32r
BF16 = mybir.dt.bfloat16
AF = mybir.ActivationFunctionType
BF16NP = ml_dtypes.bfloat16

N_CORES = 8
B, LQ, LKV, E, H, D = 2, 2048, 2048, 1024, 16, 64
HC = H // N_CORES  # heads per core = 2
JC = HC * D  # feature slice per core = 128
T = B * LQ  # 4096 query tokens
NEC = E // 128  # 8 e-chunks
NQT = LQ // 512  # 4 q tiles per batch
NTT = B * NQT  # 8 token tiles of 512
NOC = E // 128  # 8 output chunks

_NC_CACHE = {}
_LAST_META = None


def build(reps=None, phases="PAO", meta=None):
    """meta = (nkt0, nkt1): number of 128-wide valid kv chunks per batch."""
    global _LAST_META
    if meta is None:
        meta = _LAST_META
    assert meta is not None, "call make_in_maps first (sets kv chunk counts)"
    nkt = meta
    key = (reps or 0, phases, meta)
    if key in _NC_CACHE:
        return _NC_CACHE[key]
    # 512-wide projection tiles per batch; attention iterates only valid chunks
    nkv5 = tuple(-(-n * 128 // 512) for n in nkt)  # ceil(nkt*128/512)
    kvw = (nkv5[0] + nkv5[1]) * 512  # packed kv width incl. padding
    kvoff = (0, nkv5[0] * 512)  # kt_sb column offset per batch
    NG = nkt[0] + nkt[1]  # total valid kv chunks
    gcoff = (0, nkt[0])

    nc = bacc.Bacc("TRN2", target_bir_lowering=False, debug=False, num_devices=N_CORES)

    xqT = nc.dram_tensor("xqT", [E, T], BF16, kind="ExternalInput").ap()
    xkT = nc.dram_tensor("xkT", [E, kvw], BF16, kind="ExternalInput").ap()
    # packed constants: wq|wk|wv|wo (each [128, 8*128]) + bv on partition 0
    cbd = nc.dram_tensor("cb", [128, 4 * E + 128], BF16, kind="ExternalInput").ap()
    # packed fp32 constants: bq | bk | mask-bias chunks
    cfd = nc.dram_tensor("cf", [128, 2 + NG], F32, kind="ExternalInput").ap()
    outT = nc.dram_tensor("outT", [E, T], BF16, kind="ExternalOutput").ap()

    from contextlib import nullcontext

    with tile.TileContext(nc) as tc, nc.allow_low_precision(reason="bf16 kernel"):
        with tc.For_i(0, reps, 1) if reps else nullcontext():
         with (
             tc.tile_pool(name="const", bufs=1) as const,
             tc.tile_pool(name="big", bufs=1) as big,
         ):
             # ---- persistent SBUF state ----
             cb_sb = const.tile([128, 4 * E + 128], BF16, tag="cb")
             nc.sync.dma_start(out=cb_sb, in_=cbd)
             cf_sb = const.tile([128, 2 + NG], F32, tag="cf")
             nc.sync.dma_start(out=cf_sb, in_=cfd)
             wq_sb = cb_sb[:, 0 * E : 1 * E].rearrange("p (ec j) -> p ec j", ec=NEC)
             wk_sb = cb_sb[:, 1 * E : 2 * E].rearrange("p (ec j) -> p ec j", ec=NEC)
             wv_sb = cb_sb[:, 2 * E : 3 * E].rearrange("p (ec j) -> p ec j", ec=NEC)
             wo_sb = cb_sb[:, 3 * E : 4 * E].rearrange("p (oc o) -> p oc o", oc=NOC)
             bv_sb = cb_sb[0:1, 4 * E : 4 * E + 128]
             bq_sb = cf_sb[:, 0:1]
             bk_sb = cf_sb[:, 1:2]
             mb_sb = cf_sb[:, 2:]
             ones1 = const.tile([1, 128], BF16, tag="ones1")
             nc.vector.memset(ones1, 1.0)
             ones_f = const.tile([1, 65], F32, tag="onesf")
             nc.vector.memset(ones_f, 1.0)
             onesc = const.tile([1, 65], F32R, tag="onesc")
             nc.vector.tensor_copy(onesc, ones_f)

             qt_sb = big.tile([128, T], BF16, tag="qt")
             kt_sb = big.tile([128, kvw], BF16, tag="kt")
             xq_sb = big.tile([128, NTT, NEC, 512], BF16, tag="xq")
             # V per chunk: [1 | h0 d0..63 | 1 | h1 d0..63]
             v_sb = big.tile([128, NG, 130], BF16, tag="v")
             nc.vector.memset(v_sb[:, :, 0:1], 1.0)
             nc.vector.memset(v_sb[:, :, 65:66], 1.0)
             ctx_sb = big.tile([128, NTT, 512], BF16, tag="ctx")

             # ---- phase P: projections ----
             if "P" in phases:
              with (
                 tc.tile_pool(name="xin", bufs=3) as xin,
                 tc.tile_pool(name="pp", bufs=2, space="PSUM") as pp,
                 tc.tile_pool(name="vp", bufs=4, space="PSUM") as vp,
             ):
                 # K^T and V from packed kv (single read of xkT)
                 for b in range(B):
                     for t5 in range(nkv5[b]):
                         toff = kvoff[b] + t5 * 512
                         xt = xin.tile([128, NEC, 512], BF16, tag="xin")
                         # Pool SWDGE queue: keeps SP free and lets next-rep
                         # input streaming overlap this rep's attention/output
                         nc.gpsimd.dma_start(
                             out=xt,
                             in_=xkT[:, toff : toff + 512].rearrange(
                                 "(ec p) t -> p ec t", p=128
                             ),
                         )
                         pt = pp.tile([128, 512], F32, tag="pp")
                         for ec in range(NEC):
                             nc.tensor.matmul(
                                 pt,
                                 wk_sb[:, ec, :],
                                 xt[:, ec, :],
                                 start=(ec == 0),
                                 stop=(ec == NEC - 1),
                             )
                         nc.scalar.activation(
                             out=kt_sb[:, toff : toff + 512],
                             in_=pt,
                             func=AF.Identity,
                             bias=bk_sb,
                             scale=1.0,
                         )
                         # V chunks within this 512-token tile, [k, d] layout
                         for ck in range(4):
                             kt_g = t5 * 4 + ck
                             if kt_g >= nkt[b]:
                                 break
                             gc = gcoff[b] + kt_g
                             vt = vp.tile([128, 128], F32, tag="vp")
                             for ec in range(NEC):
                                 nc.tensor.matmul(
                                     vt,
                                     xt[:, ec, ck * 128 : (ck + 1) * 128],
                                     wv_sb[:, ec, :],
                                     start=(ec == 0),
                                     stop=False,
                                 )
                             nc.tensor.matmul(
                                 vt, ones1, bv_sb, start=False, stop=True
                             )
                             # cols (1:65, 66:130) <- PSUM cols 0:128
                             nc.vector.tensor_copy(
                                 v_sb[:, gc, :].rearrange(
                                     "p (a d) -> p a d", d=65
                                 )[:, :, 1:65],
                                 vt.rearrange("p (a d) -> p a d", a=2),
                             )

                 # Q^T projection: tiles 0-1 here; the rest are interleaved
                 # into the attention PE stream (phase A)
                 for tt in range(NTT):
                     nc.gpsimd.dma_start(
                         out=xq_sb[:, tt],
                         in_=xqT[:, tt * 512 : (tt + 1) * 512].rearrange(
                             "(ec p) t -> p ec t", p=128
                         ),
                     )
                 for tt in range(2):
                     pt = pp.tile([128, 512], F32, tag="pp")
                     for ec in range(NEC):
                         nc.tensor.matmul(
                             pt,
                             wq_sb[:, ec, :],
                             xq_sb[:, tt, ec, :],
                             start=(ec == 0),
                             stop=(ec == NEC - 1),
                         )
                     nc.scalar.activation(
                         out=qt_sb[:, tt * 512 : (tt + 1) * 512],
                         in_=pt,
                         func=AF.Identity,
                         bias=bq_sb,
                         scale=1.0,
                     )

             # ---- phase A: attention (software-pipelined) ----
             if "A" in phases:
              with (
                 tc.tile_pool(name="attps", bufs=2, space="PSUM") as attps,
                 tc.tile_pool(name="cxps", bufs=1, space="PSUM") as cxps,
                 tc.tile_pool(name="qpp", bufs=1, space="PSUM") as qpp,
                 tc.tile_pool(name="expm", bufs=3) as expm,
                 tc.tile_pool(name="dv", bufs=4) as dv,
             ):

                 def scores(b, q0, kt):
                     k0 = kvoff[b] + kt * 128
                     sct = attps.tile([128, 2, 512], F32, tag="sc")
                     for h in range(HC):
                         nc.tensor.matmul(
                             sct[:, h, :],
                             kt_sb[h * 64 : (h + 1) * 64, k0 : k0 + 128],
                             qt_sb[h * 64 : (h + 1) * 64, q0 : q0 + 512],
                             start=True,
                             stop=True,
                         )
                     return sct

                 def div_flush(pend):
                     # bct matmul + normalize + ctx writeback for a finished
                     # query tile; emitted after the next tile's first scores
                     # so the PE keeps streaming.
                     cxs, rrs, s2s, tt = pend
                     for h in range(HC):
                         bct = cxps.tile([65, 512], F32, tag="bct")
                         nc.tensor.matmul(bct, onesc, rrs[h], start=True, stop=True)
                         cs = dv.tile([65, 512], BF16, tag="cs")
                         nc.vector.tensor_mul(cs, s2s[h], bct)
                         nc.gpsimd.dma_start(
                             out=ctx_sb[h * 64 : (h + 1) * 64, tt, :],
                             in_=cs[1:65, :],
                         )

                 pend = None
                 qproj = []  # deferred matmuls of the in-flight Q projection
                 for b in range(B):
                     for qt in range(NQT):
                         q0 = b * LQ + qt * 512
                         tt = b * NQT + qt
                         # set up interleaved projection of q tile tt+2
                         tq = tt + 2
                         if tq < NTT:
                             qp = qpp.tile([128, 512], F32, tag="qp", name=f"qp{tq}")

                             def qp_mm(ec, qp=qp, tq=tq):
                                 nc.tensor.matmul(
                                     qp,
                                     wq_sb[:, ec, :],
                                     xq_sb[:, tq, ec, :],
                                     start=(ec == 0),
                                     stop=(ec == NEC - 1),
                                 )

                             qproj = [(qp_mm, ec) for ec in range(NEC)]
                             qp_fin = (qp, tq)
                         cxs = [
                             cxps.tile([65, 512], F32, tag=f"cx{h}", name=f"cx{h}_{b}_{qt}")
                             for h in range(HC)
                         ]
                         sct = scores(b, q0, 0)
                         if pend is not None:
                             div_flush(pend)
                             pend = None
                         for kt in range(nkt[b]):
                             gc = gcoff[b] + kt
                             emt = expm.tile([128, 2, 512], BF16, tag="expm")
                             nc.scalar.activation(
                                 out=emt.rearrange("p a t -> p (a t)"),
                                 in_=sct.rearrange("p a t -> p (a t)"),
                                 func=AF.Exp,
                                 bias=mb_sb[:, gc : gc + 1],
                                 scale=0.125,
                             )
                             if kt + 1 < nkt[b]:
                                 sct = scores(b, q0, kt + 1)
                             if qproj:
                                 fn, ec = qproj.pop(0)
                                 fn(ec)
                             st, sp = (kt == 0), (kt == nkt[b] - 1)
                             for h in range(HC):
                                 nc.tensor.matmul(
                                     cxs[h],
                                     v_sb[:, gc, h * 65 : (h + 1) * 65],
                                     emt[:, h, :],
                                     start=st,
                                     stop=sp,
                                 )
                         for fn, ec in qproj:  # leftovers (sparse masks)
                             fn(ec)
                         qproj = []
                         if tq < NTT:
                             qp, tqf = qp_fin
                             nc.vector.tensor_scalar_add(
                                 qt_sb[:, tqf * 512 : (tqf + 1) * 512], qp, bq_sb
                             )
                         rrs, s2s = [], []
                         for h in range(HC):
                             rr = dv.tile([1, 512], F32R, tag="rr")
                             nc.vector.reciprocal(rr, cxs[h][0:1, :])
                             rrs.append(rr)
                             s2 = dv.tile([65, 512], F32R, tag=f"s2{h}")
                             nc.vector.tensor_copy(s2, cxs[h])
                             s2s.append(s2)
                         pend = (cxs, rrs, s2s, tt)
                 div_flush(pend)

             # ---- phase O: output projection (partial; host sums cores) ----
             # oc-major: one 512KB DMA per output row-chunk, alternating
             # SP / Pool queues; copies alternate ACT / DVE.
             if "O" in phases:
              with (
                 tc.tile_pool(name="ops", bufs=3, space="PSUM") as ops,
                 tc.tile_pool(name="outsb", bufs=2) as outsb,
             ):
                 for oc in range(NOC):
                     ob = outsb.tile([128, NTT, 512], BF16, tag="ob")
                     for tt2 in range(NTT // 2):
                         opt = ops.tile([128, 2, 512], F32, tag="op")
                         for half in range(2):
                             nc.tensor.matmul(
                                 opt[:, half, :],
                                 wo_sb[:, oc, :],
                                 ctx_sb[:, tt2 * 2 + half, :],
                                 start=True,
                                 stop=True,
                             )
                         obp = ob[:, tt2 * 2 : tt2 * 2 + 2, :]
                         if tt2 % 2 == 0:
                             nc.scalar.activation(
                                 out=obp.rearrange("p a t -> p (a t)"),
                                 in_=opt.rearrange("p a t -> p (a t)"),
                                 func=AF.Identity,
                                 scale=1.0,
                             )
                         else:
                             nc.vector.tensor_copy(obp, opt)
                     eng = nc.sync if oc % 2 == 0 else nc.gpsimd
                     eng.dma_start(
                         out=outT[oc * 128 : (oc + 1) * 128, :],
                         in_=ob.rearrange("p a t -> p (a t)"),
                     )

    nc.compile()
    _NC_CACHE[key] = nc
    return nc


def make_in_maps(query, key_value, mask, Wq, bq, Wk, bk, Wv, bv, Wo, bo):
    global _LAST_META
    # pack valid kv positions per batch (mask==0 rows contribute exactly 0)
    idx = [np.nonzero(mask[b] != 0)[0] for b in range(B)]
    cnt = [len(i) for i in idx]
    nkt = tuple(-(-c // 128) for c in cnt)  # valid 128-chunks per batch
    nkv5 = tuple(-(-n * 128 // 512) for n in nkt)
    _LAST_META = nkt
    NG = nkt[0] + nkt[1]

    xq = np.ascontiguousarray(query.reshape(T, E).T).astype(BF16NP)
    packs = []
    for b in range(B):
        xb = np.zeros((nkv5[b] * 512, E), np.float32)
        xb[: cnt[b]] = key_value[b][idx[b]]
        packs.append(xb)
    xk = np.ascontiguousarray(np.concatenate(packs, axis=0).T).astype(BF16NP)
    # mask bias per chunk lane: 0 for valid, -1e5 for pad
    mb = np.full((128, NG), -1.0e5, np.float32)
    g = 0
    for b in range(B):
        for k in range(nkt[b]):
            valid = min(cnt[b] - k * 128, 128)
            mb[:valid, g] = 0.0
            g += 1

    def wslice(W, sl):
        # W[sl,:].T laid out [p, (ec j)]: wT[(ec p), j] -> [p, ec, j]
        wT = np.ascontiguousarray(W[sl, :].T)
        return wT.reshape(NEC, 128, JC).transpose(1, 0, 2).reshape(128, E)

    in_maps = []
    for c in range(N_CORES):
        sl = slice(c * JC, (c + 1) * JC)
        cb = np.zeros((128, 4 * E + 128), np.float32)
        cb[:, 0 * E : 1 * E] = wslice(Wq, sl)
        cb[:, 1 * E : 2 * E] = wslice(Wk, sl)
        cb[:, 2 * E : 3 * E] = wslice(Wv, sl)
        cb[:, 3 * E : 4 * E] = np.ascontiguousarray(Wo[:, sl].T)
        cb[0, 4 * E : 4 * E + 128] = bv[sl]
        cf = np.zeros((128, 2 + NG), np.float32)
        cf[:, 0] = bq[sl]
        cf[:, 1] = bk[sl]
        cf[:, 2:] = mb
        in_maps.append(
            {
                "xqT": xq,
                "xkT": xk,
                "cb": cb.astype(BF16NP),
                "cf": cf,
            }
        )
    return in_maps


def kernel(query, key_value, mask, Wq, bq, Wk, bk, Wv, bv, Wo, bo):
    in_maps = make_in_maps(
        np.asarray(query), np.asarray(key_value), np.asarray(mask),
        np.asarray(Wq), np.asarray(bq), np.asarray(Wk), np.asarray(bk),
        np.asarray(Wv), np.asarray(bv), np.asarray(Wo), np.asarray(bo),
    )
    nc = build()
    res = run_bass_kernel_spmd(nc, in_maps, list(range(N_CORES)))
    acc = np.zeros((E, T), np.float32)
    for c in range(N_CORES):
        acc += res.results[c]["outT"].astype(np.float32)
    out = np.ascontiguousarray(acc.T).reshape(B, LQ, E)
    out += np.asarray(bo, np.float32)[None, None, :]
    return out.astype(np.float32)
